# revision 7
# baseline (speedup 1.0000x reference)
"""Trainium2 Bass kernel for nn_CasparLayer (embedding -> GRU(reset_after) -> dense).

Problem shapes: B=128, T=256, VOCAB=41, EMB=512, HID=1024.

Strategy (per NeuronCore, SPMD x8, 4-way data parallel over batch):
  - Recurrent weight streaming on the PE with 4 column-tile groups
    (tile_position=(0,32j)), each holding the same stationary h chunk and
    streaming its own quarter of the weight columns. Per hidden chunk k the
    [z|r] sections stream as one fused N=512 matmul and the h section as
    N=256 (fewer instructions / LDWEIGHTS than per-gate MMs).
  - Embedding fused into the z/r stream as a one-hot K=41 matmul; the
    h-gate input projection xh = gcat_h[x] is precomputed on HOST and DMA'd
    (it sits under a DVE add, not a PE injection).
  - Keras masking folded into the gates: z' = sigmoid(zpre + 30*(1-m)) == 1
    on padded steps (h carries over); zc' = sigmoid(-zpre - 30*(1-m)) == 1-z'.
    Blend is h_new = zc*hh + z*h_prev (zc precomputed on ACT during stream).
  - h_new (F-layout [128,256]: partition=32*(h//256)+b) is PE-transposed in
    two 128x128 halves into per-step slots of two big SBUF buffers
    (hT_even/hT_odd) that double as the input to the dense head.
  - Dense head runs as batched weight-stationary GEMM windows (8 steps ->
    N=256 free) interleaved into the recurrence every 8 steps, filling the
    PE idle gap while the elementwise chain runs (keeps the HAM clock warm).

The harness contract: kernel(**inputs) takes full unsharded numpy inputs and
returns the full [128, 256, 41] float32 logits.
"""

import contextlib
import ctypes
import os
import sys
import types

sys.path.insert(0, "/opt/trn_rl_repo")

import numpy as np
import ml_dtypes

import bass_rust
import concourse.bass as bass
import concourse.tile as tile
from concourse import mybir

B = 128
T = 256
VOCAB = 41
EMB = 512
HID = 1024
H3 = 3 * HID
N_CORES = 8
BQ = 32   # batch quarter per core (4-way data parallel, x2 replicas)
Q = 4     # PE column groups = hidden quarters
KC = HID // 128  # 8 hidden-contraction chunks
HQ = HID // Q    # 256 columns per group section
WIN = 8   # xh/onehot SBUF prefetch window (steps)
DW = 8    # dense-head window (steps per batched GEMM)

F32 = mybir.dt.float32
BF16 = mybir.dt.bfloat16
AF = mybir.ActivationFunctionType


# ---------------------------------------------------------------------------
# Workaround: this walrus build accepts at most ONE sync wait per instruction;
# Tile attaches several. Hoist extras onto single-wait NOPs inserted before.
# ---------------------------------------------------------------------------
def _split_multiwaits(nc, max_waits: int = 1) -> int:
    n_split = 0
    for fn in nc.m.functions:
        for blk in fn.blocks:
            insts = blk.instructions
            i = 0
            while i < len(insts):
                ins = insts[i]
                si = ins.sync_info
                if si is not None and len(si.on_wait) > max_waits:
                    waits = list(si.on_wait)
                    keep = waits[-max_waits:]
                    hoist = waits[:-max_waits]
                    ins.sync_info = bass_rust.SyncInfo(
                        on_wait=keep, on_update=list(si.on_update)
                    )
                    for w in hoist:
                        nop = mybir.InstNoOp(
                            name=nc.get_next_instruction_name(),
                            sync_info=bass_rust.SyncInfo(on_wait=[w], on_update=[]),
                            bass_nofuse=True,
                            engine=ins.engine,
                            text_hint="wait_split",
                        )
                        nc.register_instruction(nop)
                        blk.instructions.insert(i, nop)
                        i += 1
                        n_split += 1
                i += 1
    return n_split


# ---------------------------------------------------------------------------
# Optional NTFF profiling under axon (the container's antenv stub lacks the
# hook registration module). Enabled via BASS_GRU_TRACE=1.
# ---------------------------------------------------------------------------
def _register_axon_profile_hook():
    so_path = "/opt/axon/libaxon_pjrt.so"
    if "antenv.axon_hooks" in sys.modules:
        return
    mod = types.ModuleType("antenv.axon_hooks")
    state = {"hook": None}
    mod.set_axon_ntff_profile_hook = lambda h: state.__setitem__("hook", h)
    mod.get_axon_ntff_profile_hook = lambda: state["hook"]
    sys.modules["antenv.axon_hooks"] = mod

    try:
        lib = ctypes.CDLL(so_path)
    except OSError:
        return
    if not hasattr(lib, "axon_start_nrt_profile"):
        return
    lib.axon_start_nrt_profile.argtypes = [
        ctypes.POINTER(ctypes.c_int64),
        ctypes.c_size_t,
    ]
    lib.axon_start_nrt_profile.restype = ctypes.c_int64
    lib.axon_stop_nrt_profile.argtypes = [ctypes.c_char_p]
    lib.axon_stop_nrt_profile.restype = ctypes.c_int64

    @contextlib.contextmanager
    def _hook_cm(output_dir, device_ids):
        import jax

        jax.devices()
        if device_ids:
            ids = (ctypes.c_int64 * len(device_ids))(*device_ids)
            rc = lib.axon_start_nrt_profile(ids, len(device_ids))
        else:
            rc = lib.axon_start_nrt_profile(None, 0)
        if rc != 0:
            raise RuntimeError(f"axon_start_nrt_profile rc={rc}")
        try:
            yield
        finally:
            n = lib.axon_stop_nrt_profile(str(output_dir).encode())
            print(f"ntff profile: {n} file(s) -> {output_dir}", file=sys.stderr)

    state["hook"] = _hook_cm

    import concourse.bass_utils as bu

    bu.upload_artifacts = lambda tmpdir: ""


# ---------------------------------------------------------------------------
# Kernel builder
# ---------------------------------------------------------------------------
def build_kernel(n_steps: int = T):
    nc = bass.Bass()

    wzr_d = nc.declare_dram_parameter("wzr", [128, KC * Q * 512], BF16, isOutput=False)
    wh_d = nc.declare_dram_parameter("wh", [128, KC * Q * 256], BF16, isOutput=False)
    gzr_d = nc.declare_dram_parameter("gzr", [VOCAB, Q * 512], BF16, isOutput=False)
    b1h_d = nc.declare_dram_parameter("b1h", [1, Q * 256], BF16, isOutput=False)
    xh_d = nc.declare_dram_parameter("xh", [n_steps, 128, 256], BF16, isOutput=False)
    oh_d = nc.declare_dram_parameter("onehot", [n_steps, VOCAB, BQ], BF16, isOutput=False)
    zb_d = nc.declare_dram_parameter("zbias", [128, n_steps], F32, isOutput=False)
    id_d = nc.declare_dram_parameter("identb", [128, 128], BF16, isOutput=False)
    dw_d = nc.declare_dram_parameter("dw", [128, KC * VOCAB], BF16, isOutput=False)
    out_d = nc.declare_dram_parameter("logits", [VOCAB, n_steps, BQ], BF16, isOutput=True)

    n_dw = (n_steps + DW - 1) // DW  # dense windows

    with tile.TileContext(nc) as tc:
        with contextlib.ExitStack() as ctx:
            singles = ctx.enter_context(tc.tile_pool(name="singles", bufs=1))
            state = ctx.enter_context(tc.tile_pool(name="state", bufs=1))
            temps = ctx.enter_context(tc.tile_pool(name="temps", bufs=2))
            outs = ctx.enter_context(tc.tile_pool(name="outs", bufs=2))
            ps_zr = ctx.enter_context(tc.tile_pool(name="ps_zr", bufs=2, space="PSUM"))
            ps_rh = ctx.enter_context(tc.tile_pool(name="ps_rh", bufs=2, space="PSUM"))
            ps_tr = ctx.enter_context(tc.tile_pool(name="ps_tr", bufs=2, space="PSUM"))
            ps_d = ctx.enter_context(tc.tile_pool(name="ps_d", bufs=2, space="PSUM"))

            # --- weights / constants resident in SBUF ---
            wzr = singles.tile([128, KC * Q * 512], BF16)
            nc.sync.dma_start(out=wzr, in_=wzr_d[:])
            wh = singles.tile([128, KC * Q * 256], BF16)
            nc.sync.dma_start(out=wh, in_=wh_d[:])
            gzr = singles.tile([VOCAB, Q * 512], BF16)
            nc.sync.dma_start(out=gzr, in_=gzr_d[:])
            b1h = singles.tile([1, Q * 256], BF16)
            nc.sync.dma_start(out=b1h, in_=b1h_d[:])
            identb = singles.tile([128, 128], BF16)
            nc.sync.dma_start(out=identb, in_=id_d[:])
            zb = singles.tile([128, n_steps], F32)
            nc.sync.dma_start(out=zb, in_=zb_d[:])
            dw = singles.tile([128, KC * VOCAB], BF16)
            nc.sync.dma_start(out=dw, in_=dw_d[:])
            ones = singles.tile([1, BQ], BF16)
            nc.vector.memset(ones, 1.0)

            # --- prefetch windows for per-step inputs ---
            win = min(WIN, n_steps)
            pd = max(1, win // 2)
            xh_w = singles.tile([128, win, 256], BF16)
            oh_w = singles.tile([VOCAB, win, BQ], BF16)
            for t in range(min(pd, n_steps)):
                nc.sync.dma_start(out=xh_w[:, t % win, :], in_=xh_d[t])
                nc.sync.dma_start(out=oh_w[:, t % win, :], in_=oh_d[t])

            # --- GRU state ---
            h_st = [
                state.tile([128, HQ], BF16, tag=f"h{i}", name=f"h{i}") for i in range(2)
            ]
            nc.vector.memset(h_st[0], 0.0)
            # per-step transposed h: even half (free cols 0:128 of h_new) and
            # odd half; chunk c stationary = hT_(c%2)[:, t, 32*(c//2):+32]
            hTe = state.tile([128, n_steps, 128], BF16, tag="hTe", name="hTe")
            hTo = state.tile([128, n_steps, 128], BF16, tag="hTo", name="hTo")

            def wzr_ap(k, j):
                return wzr[:, (k * Q + j) * 512 : (k * Q + j + 1) * 512]

            def wh_ap(k, j):
                return wh[:, (k * Q + j) * 256 : (k * Q + j + 1) * 256]

            def emit_dense(w):
                # batched dense head for steps [w*DW, w*DW+DW)
                t0 = w * DW
                nsteps_w = min(DW, n_steps - t0)
                nfree = nsteps_w * BQ
                dps = ps_d.tile([VOCAB, DW * BQ], F32, tag="dps", name=f"dps{w}")
                for k in range(KC):
                    src = hTe if k % 2 == 0 else hTo
                    qq = k // 2
                    nc.tensor.matmul(
                        dps[:, :nfree],
                        dw[:, k * VOCAB : (k + 1) * VOCAB],
                        src[:, t0 : t0 + nsteps_w, 32 * qq : 32 * (qq + 1)],
                        start=(k == 0),
                        stop=(k == KC - 1),
                    )
                lg = outs.tile([VOCAB, DW * BQ], BF16, tag="lg")
                if w % 2 == 0:
                    nc.scalar.copy(lg[:, :nfree], dps[:, :nfree])
                else:
                    nc.vector.tensor_copy(lg[:, :nfree], dps[:, :nfree])
                nc.sync.dma_start(
                    out=out_d[:, t0 : t0 + nsteps_w, :],
                    in_=lg[:, :nfree],
                )

            def alloc_and_inject(t):
                # input injections for step t (no dependency on h_{t-1}):
                # emitted one step ahead so they fill the PE idle gap while
                # step t-1's elementwise chain runs
                zr_ps = ps_zr.tile([128, 512], F32, tag="zr", name=f"zr{t}")
                rh_ps = ps_rh.tile([128, HQ], F32, tag="rh", name=f"rh{t}")
                oh_t = oh_w[:, t % win, :]
                for j in range(Q):
                    nc.tensor.matmul(
                        zr_ps[32 * j : 32 * (j + 1), :],
                        oh_t,
                        gzr[:, j * 512 : (j + 1) * 512],
                        start=True,
                        stop=(t == 0),
                        tile_position=(0, 32 * j),
                    )
                for j in range(Q):
                    nc.tensor.matmul(
                        rh_ps[32 * j : 32 * (j + 1), :],
                        ones,
                        b1h[:, j * 256 : (j + 1) * 256],
                        start=True,
                        stop=(t == 0),
                        tile_position=(0, 32 * j),
                    )
                return zr_ps, rh_ps

            prev_h = None  # h_new of previous step, pending transpose
            zr_cur, rh_cur = alloc_and_inject(0)
            CHUNKS = [0, 2, 4, 6, 1, 3, 5, 7]  # evens first (hTe copied first)

            for t in range(n_steps):
                h_prev = h_st[t % 2]
                h_new = h_st[(t + 1) % 2]
                zr_ps, rh_ps = zr_cur, rh_cur

                if t + pd < n_steps:
                    nc.sync.dma_start(out=xh_w[:, (t + pd) % win, :], in_=xh_d[t + pd])
                    nc.sync.dma_start(out=oh_w[:, (t + pd) % win, :], in_=oh_d[t + pd])

                # --- deferred transpose of h_{t-1} into hTe/hTo[t-1] ---
                if prev_h is not None:
                    tr0 = ps_tr.tile([128, 128], BF16, tag="tr", name=f"tr{t}e")
                    nc.tensor.transpose(tr0, prev_h[:, :128], identb)
                    tr1 = ps_tr.tile([128, 128], BF16, tag="tr", name=f"tr{t}o")
                    nc.tensor.transpose(tr1, prev_h[:, 128:], identb)
                    nc.vector.tensor_copy(hTe[:, t - 1, :], tr0)
                    nc.vector.tensor_copy(hTo[:, t - 1, :], tr1)

                # --- recurrent weight streams ---
                if t > 0:
                    for ki, k in enumerate(CHUNKS):
                        src = hTe if k % 2 == 0 else hTo
                        hs = src[:, t - 1, 32 * (k // 2) : 32 * (k // 2 + 1)]
                        for j in range(Q):
                            nc.tensor.matmul(
                                zr_ps[32 * j : 32 * (j + 1), :],
                                hs,
                                wzr_ap(k, j),
                                start=False,
                                stop=(ki == KC - 1),
                                tile_position=(0, 32 * j),
                            )
                    for ki, k in enumerate(CHUNKS):
                        src = hTe if k % 2 == 0 else hTo
                        hs = src[:, t - 1, 32 * (k // 2) : 32 * (k // 2 + 1)]
                        for j in range(Q):
                            nc.tensor.matmul(
                                rh_ps[32 * j : 32 * (j + 1), :],
                                hs,
                                wh_ap(k, j),
                                start=False,
                                stop=(ki == KC - 1),
                                tile_position=(0, 32 * j),
                            )

                # --- next step's injections: right after this stream on the
                # PE FIFO, so they run during this step's chain ---
                if t + 1 < n_steps:
                    zr_cur, rh_cur = alloc_and_inject(t + 1)

                # --- interleaved dense window (fills the PE idle gap) ---
                if t >= DW + 1 and (t - 1) % DW == 0:
                    emit_dense((t - 1) // DW - 1)

                # --- elementwise chain ---
                # ACT: r, z sigmoids (run during the rh stream), then tanh halves
                r_t = temps.tile([128, HQ], BF16, tag="r")
                nc.scalar.activation(r_t, zr_ps[:, 256:512], AF.Sigmoid)
                z_t = temps.tile([128, HQ], BF16, tag="z")
                nc.scalar.activation(
                    z_t, zr_ps[:, :256], AF.Sigmoid, bias=zb[:, t : t + 1]
                )
                # GPSIMD: zc = 1 - z, zh = z * h_prev (off the DVE/ACT queues)
                zc_t = temps.tile([128, HQ], BF16, tag="zc")
                nc.gpsimd.tensor_scalar(
                    zc_t, z_t, -1.0, 1.0, mybir.AluOpType.mult, mybir.AluOpType.add
                )
                zh_t = temps.tile([128, HQ], BF16, tag="zh")
                nc.gpsimd.tensor_mul(zh_t, z_t, h_prev)
                # DVE halves: a1 = r*rh (psum), a2 = a1 + xh
                a1_t = temps.tile([128, HQ], BF16, tag="a1")
                a2_t = temps.tile([128, HQ], BF16, tag="a2")
                for lo in (0, 128):
                    sl = slice(lo, lo + 128)
                    nc.vector.tensor_mul(a1_t[:, sl], r_t[:, sl], rh_ps[:, sl])
                    nc.vector.tensor_add(
                        a2_t[:, sl], a1_t[:, sl], xh_w[:, t % win, sl]
                    )
                hh_t = temps.tile([128, HQ], BF16, tag="hh")
                for lo in (0, 128):
                    sl = slice(lo, lo + 128)
                    nc.scalar.activation(hh_t[:, sl], a2_t[:, sl], AF.Tanh)
                u_t = temps.tile([128, HQ], BF16, tag="u")
                for lo in (0, 128):
                    sl = slice(lo, lo + 128)
                    nc.vector.tensor_mul(u_t[:, sl], zc_t[:, sl], hh_t[:, sl])
                    nc.vector.tensor_add(h_new[:, sl], u_t[:, sl], zh_t[:, sl])

                prev_h = h_new

            # --- epilogue: last transpose + remaining dense windows ---
            tr0 = ps_tr.tile([128, 128], BF16, tag="tr", name="trLe")
            nc.tensor.transpose(tr0, prev_h[:, :128], identb)
            tr1 = ps_tr.tile([128, 128], BF16, tag="tr", name="trLo")
            nc.tensor.transpose(tr1, prev_h[:, 128:], identb)
            nc.vector.tensor_copy(hTe[:, n_steps - 1, :], tr0)
            nc.vector.tensor_copy(hTo[:, n_steps - 1, :], tr1)

            done = ((n_steps - 2) // DW - 1) + 1 if n_steps >= DW + 2 else 0
            for w in range(max(0, done), n_dw):
                emit_dense(w)

    _split_multiwaits(nc)
    return nc


# ---------------------------------------------------------------------------
# Host-side prep + run
# ---------------------------------------------------------------------------
_CACHE = {}


def _prep_inputs(x, padding_mask, emb_table, gru_kernel, gru_rec_kernel, gru_bias,
                 dense_w, dense_b, n_steps):
    x = np.asarray(x)
    padding_mask = np.asarray(padding_mask)
    emb_table = np.asarray(emb_table, dtype=np.float32)
    gru_kernel = np.asarray(gru_kernel, dtype=np.float32)
    W = np.asarray(gru_rec_kernel, dtype=np.float32)
    gru_bias = np.asarray(gru_bias, dtype=np.float32)
    dense_w = np.asarray(dense_w, dtype=np.float32)

    g = emb_table @ gru_kernel + gru_bias[0][None, :]   # [VOCAB, 3H], b0 folded
    g[:, : 2 * HID] += gru_bias[1][None, : 2 * HID]     # b1 folded for z, r
    gh = np.ascontiguousarray(g[:, 2 * HID :])          # [VOCAB, H] (b0h only)

    # fused [z|r] weight quarters: wzr[p, k, j, 0:256]=Wz, [256:512]=Wr
    Wz = W[:, :HID].reshape(HID, Q, HQ)
    Wr = W[:, HID : 2 * HID].reshape(HID, Q, HQ)
    wzr = np.concatenate([Wz, Wr], axis=2)              # [H, Q, 512]
    wzr = wzr.reshape(KC, 128, Q, 512).transpose(1, 0, 2, 3)
    whm = W[:, 2 * HID :].reshape(HID, Q, HQ)
    whm = whm.reshape(KC, 128, Q, HQ).transpose(1, 0, 2, 3)

    gz = g[:, :HID].reshape(VOCAB, Q, HQ)
    gr = g[:, HID : 2 * HID].reshape(VOCAB, Q, HQ)
    gzr = np.concatenate([gz, gr], axis=2)              # [VOCAB, Q, 512]

    b1h = gru_bias[1][2 * HID :].reshape(1, Q * HQ)

    dwp = dense_w.reshape(KC, 128, VOCAB).transpose(1, 0, 2)  # [128, KC, V]

    shared = {
        "wzr": np.ascontiguousarray(wzr.reshape(128, -1)).astype(ml_dtypes.bfloat16),
        "wh": np.ascontiguousarray(whm.reshape(128, -1)).astype(ml_dtypes.bfloat16),
        "gzr": np.ascontiguousarray(gzr.reshape(VOCAB, -1)).astype(ml_dtypes.bfloat16),
        "b1h": np.ascontiguousarray(b1h).astype(ml_dtypes.bfloat16),
        "identb": np.eye(128, dtype=np.float32).astype(ml_dtypes.bfloat16),
        "dw": np.ascontiguousarray(dwp.reshape(128, -1)).astype(ml_dtypes.bfloat16),
    }

    gh_bf = gh.astype(ml_dtypes.bfloat16)

    in_maps = []
    for c in range(N_CORES):
        q = c % Q
        xs = x[q * BQ : (q + 1) * BQ]                   # [BQ, T]
        ms = padding_mask[q * BQ : (q + 1) * BQ]

        # xh[t, p=32*qq+b, f] = gh[x[b,t], 256*qq + f]  (F-layout)
        e = gh_bf[xs[:, :n_steps]]                      # [BQ, T, H] bf16
        e = e.transpose(1, 0, 2).reshape(n_steps, BQ, Q, HQ)
        e = e.transpose(0, 2, 1, 3).reshape(n_steps, 128, HQ)

        onehot = np.zeros((n_steps, VOCAB, BQ), dtype=np.float32)
        tt = np.arange(n_steps)
        for b in range(BQ):
            onehot[tt, xs[b, :n_steps], b] = 1.0
        zbias = np.where(ms[:, :n_steps], 0.0, 30.0).astype(np.float32)  # [BQ, T]
        zbias = np.tile(zbias, (128 // BQ, 1))          # F-layout partitions
        in_maps.append(
            dict(
                shared,
                xh=np.ascontiguousarray(e),
                onehot=onehot.astype(ml_dtypes.bfloat16),
                zbias=np.ascontiguousarray(zbias),
            )
        )
    return in_maps


def kernel(x, padding_mask, emb_table, gru_kernel, gru_rec_kernel, gru_bias,
           dense_w, dense_b, _n_steps: int = T):
    from concourse.bass_utils import run_bass_kernel_spmd

    trace = os.environ.get("BASS_GRU_TRACE", "") == "1"
    if trace:
        _register_axon_profile_hook()

    n_steps = _n_steps
    if n_steps not in _CACHE:
        _CACHE[n_steps] = build_kernel(n_steps)
    nc = _CACHE[n_steps]

    in_maps = _prep_inputs(x, padding_mask, emb_table, gru_kernel, gru_rec_kernel,
                           gru_bias, dense_w, dense_b, n_steps)
    res = run_bass_kernel_spmd(nc, in_maps, list(range(N_CORES)), trace=trace)
    if trace:
        kernel.last_exec_time_ns = res.exec_time_ns
        print(f"HW exec time: {res.exec_time_ns} ns")

    db = np.asarray(dense_b, dtype=np.float32)
    out = np.empty((B, n_steps, VOCAB), dtype=np.float32)
    for q in range(Q):
        lg = np.asarray(res.results[q]["logits"], dtype=np.float32)  # [V, T, BQ]
        out[q * BQ : (q + 1) * BQ] = lg.transpose(2, 1, 0)
    out += db[None, None, :]
    return np.ascontiguousarray(out)


kernel.last_exec_time_ns = None


# revision 8
# speedup vs baseline: 1.2109x; 1.2109x over previous
"""Trainium2 Bass kernel for nn_CasparLayer (embedding -> GRU(reset_after) -> dense).

Problem shapes: B=128, T=256, VOCAB=41, EMB=512, HID=1024.

Strategy (per NeuronCore, SPMD x8, 4-way data parallel over batch):
  - Recurrent weight streaming on the PE with 4 column-tile groups
    (tile_position=(0,32j)), each holding the same stationary h chunk and
    streaming its own quarter of the weight columns. Per hidden chunk k the
    [z|r] sections stream as one fused N=512 matmul and the h section as
    N=256 (fewer instructions / LDWEIGHTS than per-gate MMs).
  - Embedding fused into the z/r stream as a one-hot K=41 matmul; the
    h-gate input projection xh = gcat_h[x] is precomputed on HOST and DMA'd
    (it sits under a DVE add, not a PE injection).
  - Keras masking folded into the gates: z' = sigmoid(zpre + 30*(1-m)) == 1
    on padded steps (h carries over); zc' = sigmoid(-zpre - 30*(1-m)) == 1-z'.
    Blend is h_new = zc*hh + z*h_prev (zc precomputed on ACT during stream).
  - h_new (F-layout [128,256]: partition=32*(h//256)+b) is PE-transposed in
    two 128x128 halves into per-step slots of two big SBUF buffers
    (hT_even/hT_odd) that double as the input to the dense head.
  - Dense head runs as batched weight-stationary GEMM windows (8 steps ->
    N=256 free) interleaved into the recurrence every 8 steps, filling the
    PE idle gap while the elementwise chain runs (keeps the HAM clock warm).

The harness contract: kernel(**inputs) takes full unsharded numpy inputs and
returns the full [128, 256, 41] float32 logits.
"""

import contextlib
import ctypes
import os
import sys
import types

sys.path.insert(0, "/opt/trn_rl_repo")

import numpy as np
import ml_dtypes

import bass_rust
import concourse.bass as bass
import concourse.tile as tile
from concourse import mybir

B = 128
T = 256
VOCAB = 41
EMB = 512
HID = 1024
H3 = 3 * HID
N_CORES = 8
BQ = 32   # batch quarter per core (4-way data parallel, x2 replicas)
Q = 4     # PE column groups = hidden quarters
KC = HID // 128  # 8 hidden-contraction chunks
HQ = HID // Q    # 256 columns per group section
WIN = 8   # xh/onehot SBUF prefetch window (steps)
DW = 8    # dense-head window (steps per batched GEMM)

F32 = mybir.dt.float32
BF16 = mybir.dt.bfloat16
AF = mybir.ActivationFunctionType


# ---------------------------------------------------------------------------
# Workaround: this walrus build accepts at most ONE sync wait per instruction;
# Tile attaches several. Hoist extras onto single-wait NOPs inserted before.
# ---------------------------------------------------------------------------
def _split_multiwaits(nc, max_waits: int = 1) -> int:
    n_split = 0
    for fn in nc.m.functions:
        for blk in fn.blocks:
            insts = blk.instructions
            i = 0
            while i < len(insts):
                ins = insts[i]
                si = ins.sync_info
                if si is not None and len(si.on_wait) > max_waits:
                    waits = list(si.on_wait)
                    keep = waits[-max_waits:]
                    hoist = waits[:-max_waits]
                    ins.sync_info = bass_rust.SyncInfo(
                        on_wait=keep, on_update=list(si.on_update)
                    )
                    for w in hoist:
                        nop = mybir.InstNoOp(
                            name=nc.get_next_instruction_name(),
                            sync_info=bass_rust.SyncInfo(on_wait=[w], on_update=[]),
                            bass_nofuse=True,
                            engine=ins.engine,
                            text_hint="wait_split",
                        )
                        nc.register_instruction(nop)
                        blk.instructions.insert(i, nop)
                        i += 1
                        n_split += 1
                i += 1
    return n_split


# ---------------------------------------------------------------------------
# Optional NTFF profiling under axon (the container's antenv stub lacks the
# hook registration module). Enabled via BASS_GRU_TRACE=1.
# ---------------------------------------------------------------------------
def _register_axon_profile_hook():
    so_path = "/opt/axon/libaxon_pjrt.so"
    if "antenv.axon_hooks" in sys.modules:
        return
    mod = types.ModuleType("antenv.axon_hooks")
    state = {"hook": None}
    mod.set_axon_ntff_profile_hook = lambda h: state.__setitem__("hook", h)
    mod.get_axon_ntff_profile_hook = lambda: state["hook"]
    sys.modules["antenv.axon_hooks"] = mod

    try:
        lib = ctypes.CDLL(so_path)
    except OSError:
        return
    if not hasattr(lib, "axon_start_nrt_profile"):
        return
    lib.axon_start_nrt_profile.argtypes = [
        ctypes.POINTER(ctypes.c_int64),
        ctypes.c_size_t,
    ]
    lib.axon_start_nrt_profile.restype = ctypes.c_int64
    lib.axon_stop_nrt_profile.argtypes = [ctypes.c_char_p]
    lib.axon_stop_nrt_profile.restype = ctypes.c_int64

    @contextlib.contextmanager
    def _hook_cm(output_dir, device_ids):
        import jax

        jax.devices()
        if device_ids:
            ids = (ctypes.c_int64 * len(device_ids))(*device_ids)
            rc = lib.axon_start_nrt_profile(ids, len(device_ids))
        else:
            rc = lib.axon_start_nrt_profile(None, 0)
        if rc != 0:
            raise RuntimeError(f"axon_start_nrt_profile rc={rc}")
        try:
            yield
        finally:
            n = lib.axon_stop_nrt_profile(str(output_dir).encode())
            print(f"ntff profile: {n} file(s) -> {output_dir}", file=sys.stderr)

    state["hook"] = _hook_cm

    import concourse.bass_utils as bu

    bu.upload_artifacts = lambda tmpdir: ""


# ---------------------------------------------------------------------------
# Kernel builder
# ---------------------------------------------------------------------------
def build_kernel(n_steps: int = T):
    nc = bass.Bass()

    wzr_d = nc.declare_dram_parameter("wzr", [128, KC * Q * 512], BF16, isOutput=False)
    wh_d = nc.declare_dram_parameter("wh", [128, KC * Q * 256], BF16, isOutput=False)
    gzr_d = nc.declare_dram_parameter("gzr", [VOCAB, Q * 512], BF16, isOutput=False)
    b1h_d = nc.declare_dram_parameter("b1h", [1, Q * 256], BF16, isOutput=False)
    xh_d = nc.declare_dram_parameter("xh", [n_steps, 128, 256], BF16, isOutput=False)
    oh_d = nc.declare_dram_parameter("onehot", [n_steps, VOCAB, BQ], BF16, isOutput=False)
    zb_d = nc.declare_dram_parameter("zbias", [128, n_steps], F32, isOutput=False)
    id_d = nc.declare_dram_parameter("identb", [128, 128], BF16, isOutput=False)
    dw_d = nc.declare_dram_parameter("dw", [128, KC * VOCAB], BF16, isOutput=False)
    out_d = nc.declare_dram_parameter("logits", [VOCAB, n_steps, BQ], BF16, isOutput=True)

    n_dw = (n_steps + DW - 1) // DW  # dense windows

    with tile.TileContext(nc) as tc:
        with contextlib.ExitStack() as ctx:
            singles = ctx.enter_context(tc.tile_pool(name="singles", bufs=1))
            state = ctx.enter_context(tc.tile_pool(name="state", bufs=1))
            temps = ctx.enter_context(tc.tile_pool(name="temps", bufs=2))
            outs = ctx.enter_context(tc.tile_pool(name="outs", bufs=2))
            ps_zr = ctx.enter_context(tc.tile_pool(name="ps_zr", bufs=2, space="PSUM"))
            ps_rh = ctx.enter_context(tc.tile_pool(name="ps_rh", bufs=2, space="PSUM"))
            ps_tr = ctx.enter_context(tc.tile_pool(name="ps_tr", bufs=2, space="PSUM"))
            ps_d = ctx.enter_context(tc.tile_pool(name="ps_d", bufs=2, space="PSUM"))

            # --- weights / constants resident in SBUF ---
            wzr = singles.tile([128, KC * Q * 512], BF16)
            nc.sync.dma_start(out=wzr, in_=wzr_d[:])
            wh = singles.tile([128, KC * Q * 256], BF16)
            nc.sync.dma_start(out=wh, in_=wh_d[:])
            gzr = singles.tile([VOCAB, Q * 512], BF16)
            nc.sync.dma_start(out=gzr, in_=gzr_d[:])
            b1h = singles.tile([1, Q * 256], BF16)
            nc.sync.dma_start(out=b1h, in_=b1h_d[:])
            identb = singles.tile([128, 128], BF16)
            nc.sync.dma_start(out=identb, in_=id_d[:])
            zb = singles.tile([128, n_steps], F32)
            nc.sync.dma_start(out=zb, in_=zb_d[:])
            dw = singles.tile([128, KC * VOCAB], BF16)
            nc.sync.dma_start(out=dw, in_=dw_d[:])
            ones = singles.tile([1, BQ], BF16)
            nc.vector.memset(ones, 1.0)

            # --- prefetch windows for per-step inputs ---
            win = min(WIN, n_steps)
            pd = max(1, win // 2)
            xh_w = singles.tile([128, win, 256], BF16)
            oh_w = singles.tile([VOCAB, win, BQ], BF16)
            for t in range(min(pd, n_steps)):
                nc.sync.dma_start(out=xh_w[:, t % win, :], in_=xh_d[t])
                nc.sync.dma_start(out=oh_w[:, t % win, :], in_=oh_d[t])

            # --- GRU state ---
            h_st = [
                state.tile([128, HQ], BF16, tag=f"h{i}", name=f"h{i}") for i in range(2)
            ]
            nc.vector.memset(h_st[0], 0.0)
            # per-step transposed h: even half (free cols 0:128 of h_new) and
            # odd half; chunk c stationary = hT_(c%2)[:, t, 32*(c//2):+32]
            hTe = state.tile([128, n_steps, 128], BF16, tag="hTe", name="hTe")
            hTo = state.tile([128, n_steps, 128], BF16, tag="hTo", name="hTo")

            def wzr_ap(k, j):
                return wzr[:, (k * Q + j) * 512 : (k * Q + j + 1) * 512]

            def wh_ap(k, j):
                return wh[:, (k * Q + j) * 256 : (k * Q + j + 1) * 256]

            def emit_dense(w):
                # batched dense head for steps [w*DW, w*DW+DW)
                t0 = w * DW
                nsteps_w = min(DW, n_steps - t0)
                nfree = nsteps_w * BQ
                dps = ps_d.tile([VOCAB, DW * BQ], F32, tag="dps", name=f"dps{w}")
                for k in range(KC):
                    src = hTe if k % 2 == 0 else hTo
                    qq = k // 2
                    nc.tensor.matmul(
                        dps[:, :nfree],
                        dw[:, k * VOCAB : (k + 1) * VOCAB],
                        src[:, t0 : t0 + nsteps_w, 32 * qq : 32 * (qq + 1)],
                        start=(k == 0),
                        stop=(k == KC - 1),
                    )
                lg = outs.tile([VOCAB, DW * BQ], BF16, tag="lg")
                if w % 2 == 0:
                    nc.scalar.copy(lg[:, :nfree], dps[:, :nfree])
                else:
                    nc.vector.tensor_copy(lg[:, :nfree], dps[:, :nfree])
                nc.sync.dma_start(
                    out=out_d[:, t0 : t0 + nsteps_w, :],
                    in_=lg[:, :nfree],
                )

            def alloc_and_inject(t):
                # input injections for step t (no dependency on h_{t-1}):
                # emitted one step ahead so they fill the PE idle gap while
                # step t-1's elementwise chain runs
                zr_ps = ps_zr.tile([128, 512], F32, tag="zr", name=f"zr{t}")
                rh_ps = ps_rh.tile([128, HQ], F32, tag="rh", name=f"rh{t}")
                oh_t = oh_w[:, t % win, :]
                for j in range(Q):
                    nc.tensor.matmul(
                        zr_ps[32 * j : 32 * (j + 1), :],
                        oh_t,
                        gzr[:, j * 512 : (j + 1) * 512],
                        start=True,
                        stop=(t == 0),
                        tile_position=(0, 32 * j),
                    )
                for j in range(Q):
                    nc.tensor.matmul(
                        rh_ps[32 * j : 32 * (j + 1), :],
                        ones,
                        b1h[:, j * 256 : (j + 1) * 256],
                        start=True,
                        stop=(t == 0),
                        tile_position=(0, 32 * j),
                    )
                return zr_ps, rh_ps

            prev_h = None  # h_new of previous step, pending transpose
            zr_cur, rh_cur = alloc_and_inject(0)
            CHUNKS = [0, 2, 4, 6, 1, 3, 5, 7]  # evens first (hTe copied first)

            for t in range(n_steps):
                h_prev = h_st[t % 2]
                h_new = h_st[(t + 1) % 2]
                zr_ps, rh_ps = zr_cur, rh_cur

                if t + pd < n_steps:
                    nc.sync.dma_start(out=xh_w[:, (t + pd) % win, :], in_=xh_d[t + pd])
                    nc.sync.dma_start(out=oh_w[:, (t + pd) % win, :], in_=oh_d[t + pd])

                # --- deferred transpose of h_{t-1} into hTe/hTo[t-1] ---
                if prev_h is not None:
                    tr0 = ps_tr.tile([128, 128], BF16, tag="tr", name=f"tr{t}e")
                    nc.tensor.transpose(tr0, prev_h[:, :128], identb)
                    tr1 = ps_tr.tile([128, 128], BF16, tag="tr", name=f"tr{t}o")
                    nc.tensor.transpose(tr1, prev_h[:, 128:], identb)
                    nc.vector.tensor_copy(hTe[:, t - 1, :], tr0)
                    nc.vector.tensor_copy(hTo[:, t - 1, :], tr1)

                # --- recurrent weight streams ---
                if t > 0:
                    for ki, k in enumerate(CHUNKS):
                        src = hTe if k % 2 == 0 else hTo
                        hs = src[:, t - 1, 32 * (k // 2) : 32 * (k // 2 + 1)]
                        for j in range(Q):
                            nc.tensor.matmul(
                                zr_ps[32 * j : 32 * (j + 1), :],
                                hs,
                                wzr_ap(k, j),
                                start=False,
                                stop=(ki == KC - 1),
                                tile_position=(0, 32 * j),
                            )
                    for ki, k in enumerate(CHUNKS):
                        src = hTe if k % 2 == 0 else hTo
                        hs = src[:, t - 1, 32 * (k // 2) : 32 * (k // 2 + 1)]
                        for j in range(Q):
                            nc.tensor.matmul(
                                rh_ps[32 * j : 32 * (j + 1), :],
                                hs,
                                wh_ap(k, j),
                                start=False,
                                stop=(ki == KC - 1),
                                tile_position=(0, 32 * j),
                            )

                # --- next step's injections: right after this stream on the
                # PE FIFO, so they run during this step's chain ---
                if t + 1 < n_steps:
                    zr_cur, rh_cur = alloc_and_inject(t + 1)

                # --- interleaved dense window (fills the PE idle gap) ---
                if t >= DW + 1 and (t - 1) % DW == 0:
                    emit_dense((t - 1) // DW - 1)

                # --- elementwise chain ---
                # ACT: r, z sigmoids (run during the rh stream), then tanh halves
                r_t = temps.tile([128, HQ], BF16, tag="r")
                nc.scalar.activation(r_t, zr_ps[:, 256:512], AF.Sigmoid)
                z_t = temps.tile([128, HQ], BF16, tag="z")
                nc.scalar.activation(
                    z_t, zr_ps[:, :256], AF.Sigmoid, bias=zb[:, t : t + 1]
                )
                # GPSIMD: zc = 1 - z, zh = z * h_prev (off the DVE/ACT queues)
                zc_t = temps.tile([128, HQ], BF16, tag="zc")
                nc.gpsimd.tensor_scalar(
                    zc_t, z_t, -1.0, 1.0, mybir.AluOpType.mult, mybir.AluOpType.add
                )
                zh_t = temps.tile([128, HQ], BF16, tag="zh")
                nc.gpsimd.tensor_mul(zh_t, z_t, h_prev)
                # DVE: a1 = r*rh (psum), a2 = a1 + xh, then blend
                a1_t = temps.tile([128, HQ], BF16, tag="a1")
                nc.vector.tensor_mul(a1_t, r_t, rh_ps)
                a2_t = temps.tile([128, HQ], BF16, tag="a2")
                nc.vector.tensor_add(a2_t, a1_t, xh_w[:, t % win, :])
                hh_t = temps.tile([128, HQ], BF16, tag="hh")
                nc.scalar.activation(hh_t, a2_t, AF.Tanh)
                u_t = temps.tile([128, HQ], BF16, tag="u")
                nc.vector.tensor_mul(u_t, zc_t, hh_t)
                nc.vector.tensor_add(h_new, u_t, zh_t)

                prev_h = h_new

            # --- epilogue: last transpose + remaining dense windows ---
            tr0 = ps_tr.tile([128, 128], BF16, tag="tr", name="trLe")
            nc.tensor.transpose(tr0, prev_h[:, :128], identb)
            tr1 = ps_tr.tile([128, 128], BF16, tag="tr", name="trLo")
            nc.tensor.transpose(tr1, prev_h[:, 128:], identb)
            nc.vector.tensor_copy(hTe[:, n_steps - 1, :], tr0)
            nc.vector.tensor_copy(hTo[:, n_steps - 1, :], tr1)

            done = ((n_steps - 2) // DW - 1) + 1 if n_steps >= DW + 2 else 0
            for w in range(max(0, done), n_dw):
                emit_dense(w)

    _split_multiwaits(nc)
    return nc


# ---------------------------------------------------------------------------
# Host-side prep + run
# ---------------------------------------------------------------------------
_CACHE = {}


def _prep_inputs(x, padding_mask, emb_table, gru_kernel, gru_rec_kernel, gru_bias,
                 dense_w, dense_b, n_steps):
    x = np.asarray(x)
    padding_mask = np.asarray(padding_mask)
    emb_table = np.asarray(emb_table, dtype=np.float32)
    gru_kernel = np.asarray(gru_kernel, dtype=np.float32)
    W = np.asarray(gru_rec_kernel, dtype=np.float32)
    gru_bias = np.asarray(gru_bias, dtype=np.float32)
    dense_w = np.asarray(dense_w, dtype=np.float32)

    g = emb_table @ gru_kernel + gru_bias[0][None, :]   # [VOCAB, 3H], b0 folded
    g[:, : 2 * HID] += gru_bias[1][None, : 2 * HID]     # b1 folded for z, r
    gh = np.ascontiguousarray(g[:, 2 * HID :])          # [VOCAB, H] (b0h only)

    # fused [z|r] weight quarters: wzr[p, k, j, 0:256]=Wz, [256:512]=Wr
    Wz = W[:, :HID].reshape(HID, Q, HQ)
    Wr = W[:, HID : 2 * HID].reshape(HID, Q, HQ)
    wzr = np.concatenate([Wz, Wr], axis=2)              # [H, Q, 512]
    wzr = wzr.reshape(KC, 128, Q, 512).transpose(1, 0, 2, 3)
    whm = W[:, 2 * HID :].reshape(HID, Q, HQ)
    whm = whm.reshape(KC, 128, Q, HQ).transpose(1, 0, 2, 3)

    gz = g[:, :HID].reshape(VOCAB, Q, HQ)
    gr = g[:, HID : 2 * HID].reshape(VOCAB, Q, HQ)
    gzr = np.concatenate([gz, gr], axis=2)              # [VOCAB, Q, 512]

    b1h = gru_bias[1][2 * HID :].reshape(1, Q * HQ)

    dwp = dense_w.reshape(KC, 128, VOCAB).transpose(1, 0, 2)  # [128, KC, V]

    shared = {
        "wzr": np.ascontiguousarray(wzr.reshape(128, -1)).astype(ml_dtypes.bfloat16),
        "wh": np.ascontiguousarray(whm.reshape(128, -1)).astype(ml_dtypes.bfloat16),
        "gzr": np.ascontiguousarray(gzr.reshape(VOCAB, -1)).astype(ml_dtypes.bfloat16),
        "b1h": np.ascontiguousarray(b1h).astype(ml_dtypes.bfloat16),
        "identb": np.eye(128, dtype=np.float32).astype(ml_dtypes.bfloat16),
        "dw": np.ascontiguousarray(dwp.reshape(128, -1)).astype(ml_dtypes.bfloat16),
    }

    gh_bf = gh.astype(ml_dtypes.bfloat16)

    in_maps = []
    for c in range(N_CORES):
        q = c % Q
        xs = x[q * BQ : (q + 1) * BQ]                   # [BQ, T]
        ms = padding_mask[q * BQ : (q + 1) * BQ]

        # xh[t, p=32*qq+b, f] = gh[x[b,t], 256*qq + f]  (F-layout)
        e = gh_bf[xs[:, :n_steps]]                      # [BQ, T, H] bf16
        e = e.transpose(1, 0, 2).reshape(n_steps, BQ, Q, HQ)
        e = e.transpose(0, 2, 1, 3).reshape(n_steps, 128, HQ)

        onehot = np.zeros((n_steps, VOCAB, BQ), dtype=np.float32)
        tt = np.arange(n_steps)
        for b in range(BQ):
            onehot[tt, xs[b, :n_steps], b] = 1.0
        zbias = np.where(ms[:, :n_steps], 0.0, 30.0).astype(np.float32)  # [BQ, T]
        zbias = np.tile(zbias, (128 // BQ, 1))          # F-layout partitions
        in_maps.append(
            dict(
                shared,
                xh=np.ascontiguousarray(e),
                onehot=onehot.astype(ml_dtypes.bfloat16),
                zbias=np.ascontiguousarray(zbias),
            )
        )
    return in_maps


def kernel(x, padding_mask, emb_table, gru_kernel, gru_rec_kernel, gru_bias,
           dense_w, dense_b, _n_steps: int = T):
    from concourse.bass_utils import run_bass_kernel_spmd

    trace = os.environ.get("BASS_GRU_TRACE", "") == "1"
    if trace:
        _register_axon_profile_hook()

    n_steps = _n_steps
    if n_steps not in _CACHE:
        _CACHE[n_steps] = build_kernel(n_steps)
    nc = _CACHE[n_steps]

    in_maps = _prep_inputs(x, padding_mask, emb_table, gru_kernel, gru_rec_kernel,
                           gru_bias, dense_w, dense_b, n_steps)
    res = run_bass_kernel_spmd(nc, in_maps, list(range(N_CORES)), trace=trace)
    if trace:
        kernel.last_exec_time_ns = res.exec_time_ns
        print(f"HW exec time: {res.exec_time_ns} ns")

    db = np.asarray(dense_b, dtype=np.float32)
    out = np.empty((B, n_steps, VOCAB), dtype=np.float32)
    for q in range(Q):
        lg = np.asarray(res.results[q]["logits"], dtype=np.float32)  # [V, T, BQ]
        out[q * BQ : (q + 1) * BQ] = lg.transpose(2, 1, 0)
    out += db[None, None, :]
    return np.ascontiguousarray(out)


kernel.last_exec_time_ns = None


# revision 20
# speedup vs baseline: 1.2221x; 1.0093x over previous
"""Trainium2 Bass kernel for nn_CasparLayer (embedding -> GRU(reset_after) -> dense).

Problem shapes: B=128, T=256, VOCAB=41, EMB=512, HID=1024.

Strategy (per NeuronCore, SPMD x8, 4-way data parallel over batch):
  - Recurrent weight streaming on the PE with 4 column-tile groups
    (tile_position=(0,32j)), each holding the same stationary h chunk and
    streaming its own quarter of the weight columns. Per hidden chunk k the
    [z|r] sections stream as one fused N=512 matmul and the h section as
    N=256 (fewer instructions / LDWEIGHTS than per-gate MMs).
  - Embedding fused into the z/r stream as a one-hot K=41 matmul; the
    h-gate input projection xh = gcat_h[x] is precomputed on HOST and DMA'd
    (it sits under a DVE add, not a PE injection).
  - Keras masking folded into the gates: z' = sigmoid(zpre + 30*(1-m)) == 1
    on padded steps (h carries over); zc' = sigmoid(-zpre - 30*(1-m)) == 1-z'.
    Blend is h_new = zc*hh + z*h_prev (zc precomputed on ACT during stream).
  - h_new (F-layout [128,256]: partition=32*(h//256)+b) is PE-transposed in
    two 128x128 halves into per-step slots of two big SBUF buffers
    (hT_even/hT_odd) that double as the input to the dense head.
  - Dense head runs as batched weight-stationary GEMM windows (8 steps ->
    N=256 free) interleaved into the recurrence every 8 steps, filling the
    PE idle gap while the elementwise chain runs (keeps the HAM clock warm).

The harness contract: kernel(**inputs) takes full unsharded numpy inputs and
returns the full [128, 256, 41] float32 logits.
"""

import contextlib
import ctypes
import os
import sys
import types

sys.path.insert(0, "/opt/trn_rl_repo")

import numpy as np
import ml_dtypes

import bass_rust
import concourse.bass as bass
import concourse.tile as tile
from concourse import mybir

B = 128
T = 256
VOCAB = 41
EMB = 512
HID = 1024
H3 = 3 * HID
N_CORES = 8
BQ = 32   # batch quarter per core (4-way data parallel, x2 replicas)
Q = 4     # PE column groups = hidden quarters
KC = HID // 128  # 8 hidden-contraction chunks
HQ = HID // Q    # 256 columns per group section
WIN = 8   # xh/onehot SBUF prefetch window (steps)
DW = 8    # dense-head window (steps per batched GEMM)

F32 = mybir.dt.float32
BF16 = mybir.dt.bfloat16
AF = mybir.ActivationFunctionType


# ---------------------------------------------------------------------------
# Workaround: this walrus build accepts at most ONE sync wait per instruction;
# Tile attaches several. Hoist extras onto single-wait NOPs inserted before.
# ---------------------------------------------------------------------------
def _split_multiwaits(nc, max_waits: int = 1) -> int:
    n_split = 0
    for fn in nc.m.functions:
        for blk in fn.blocks:
            insts = blk.instructions
            i = 0
            while i < len(insts):
                ins = insts[i]
                si = ins.sync_info
                if si is not None and len(si.on_wait) > max_waits:
                    waits = list(si.on_wait)
                    keep = waits[-max_waits:]
                    hoist = waits[:-max_waits]
                    ins.sync_info = bass_rust.SyncInfo(
                        on_wait=keep, on_update=list(si.on_update)
                    )
                    for w in hoist:
                        nop = mybir.InstNoOp(
                            name=nc.get_next_instruction_name(),
                            sync_info=bass_rust.SyncInfo(on_wait=[w], on_update=[]),
                            bass_nofuse=True,
                            engine=ins.engine,
                            text_hint="wait_split",
                        )
                        nc.register_instruction(nop)
                        blk.instructions.insert(i, nop)
                        i += 1
                        n_split += 1
                i += 1
    return n_split


# ---------------------------------------------------------------------------
# Optional NTFF profiling under axon (the container's antenv stub lacks the
# hook registration module). Enabled via BASS_GRU_TRACE=1.
# ---------------------------------------------------------------------------
def _register_axon_profile_hook():
    so_path = "/opt/axon/libaxon_pjrt.so"
    if "antenv.axon_hooks" in sys.modules:
        return
    mod = types.ModuleType("antenv.axon_hooks")
    state = {"hook": None}
    mod.set_axon_ntff_profile_hook = lambda h: state.__setitem__("hook", h)
    mod.get_axon_ntff_profile_hook = lambda: state["hook"]
    sys.modules["antenv.axon_hooks"] = mod

    try:
        lib = ctypes.CDLL(so_path)
    except OSError:
        return
    if not hasattr(lib, "axon_start_nrt_profile"):
        return
    lib.axon_start_nrt_profile.argtypes = [
        ctypes.POINTER(ctypes.c_int64),
        ctypes.c_size_t,
    ]
    lib.axon_start_nrt_profile.restype = ctypes.c_int64
    lib.axon_stop_nrt_profile.argtypes = [ctypes.c_char_p]
    lib.axon_stop_nrt_profile.restype = ctypes.c_int64

    @contextlib.contextmanager
    def _hook_cm(output_dir, device_ids):
        import jax

        jax.devices()
        if device_ids:
            ids = (ctypes.c_int64 * len(device_ids))(*device_ids)
            rc = lib.axon_start_nrt_profile(ids, len(device_ids))
        else:
            rc = lib.axon_start_nrt_profile(None, 0)
        if rc != 0:
            raise RuntimeError(f"axon_start_nrt_profile rc={rc}")
        try:
            yield
        finally:
            n = lib.axon_stop_nrt_profile(str(output_dir).encode())
            print(f"ntff profile: {n} file(s) -> {output_dir}", file=sys.stderr)

    state["hook"] = _hook_cm

    import concourse.bass_utils as bu

    bu.upload_artifacts = lambda tmpdir: ""


# ---------------------------------------------------------------------------
# Kernel builder
# ---------------------------------------------------------------------------
def build_kernel(n_steps: int = T):
    nc = bass.Bass()

    wzr_d = nc.declare_dram_parameter("wzr", [128, KC * Q * 512], BF16, isOutput=False)
    wh_d = nc.declare_dram_parameter("wh", [128, KC * Q * 256], BF16, isOutput=False)
    gzr_d = nc.declare_dram_parameter("gzr", [VOCAB, Q * 512], BF16, isOutput=False)
    b1h_d = nc.declare_dram_parameter("b1h", [1, Q * 256], BF16, isOutput=False)
    xh_d = nc.declare_dram_parameter("xh", [n_steps, 128, 256], BF16, isOutput=False)
    oh_d = nc.declare_dram_parameter("onehot", [n_steps, VOCAB, BQ], BF16, isOutput=False)
    zb_d = nc.declare_dram_parameter("zbias", [128, n_steps], F32, isOutput=False)
    id_d = nc.declare_dram_parameter("identb", [128, 128], BF16, isOutput=False)
    dw_d = nc.declare_dram_parameter("dw", [128, KC * VOCAB], BF16, isOutput=False)
    out_d = nc.declare_dram_parameter("logits", [VOCAB, n_steps, BQ], BF16, isOutput=True)

    n_dw = (n_steps + DW - 1) // DW  # dense windows

    with tile.TileContext(nc) as tc:
        with contextlib.ExitStack() as ctx:
            singles = ctx.enter_context(tc.tile_pool(name="singles", bufs=1))
            state = ctx.enter_context(tc.tile_pool(name="state", bufs=1))
            temps = ctx.enter_context(tc.tile_pool(name="temps", bufs=2))
            outs = ctx.enter_context(tc.tile_pool(name="outs", bufs=2))
            ps_zr = ctx.enter_context(tc.tile_pool(name="ps_zr", bufs=2, space="PSUM"))
            ps_rh = ctx.enter_context(tc.tile_pool(name="ps_rh", bufs=2, space="PSUM"))
            ps_tr = ctx.enter_context(tc.tile_pool(name="ps_tr", bufs=2, space="PSUM"))
            ps_d = ctx.enter_context(tc.tile_pool(name="ps_d", bufs=2, space="PSUM"))

            # --- weights / constants resident in SBUF ---
            wzr = singles.tile([128, KC * Q * 512], BF16)
            nc.sync.dma_start(out=wzr, in_=wzr_d[:])
            wh = singles.tile([128, KC * Q * 256], BF16)
            nc.sync.dma_start(out=wh, in_=wh_d[:])
            gzr = singles.tile([VOCAB, Q * 512], BF16)
            nc.sync.dma_start(out=gzr, in_=gzr_d[:])
            b1h = singles.tile([1, Q * 256], BF16)
            nc.sync.dma_start(out=b1h, in_=b1h_d[:])
            identb = singles.tile([128, 128], BF16)
            nc.sync.dma_start(out=identb, in_=id_d[:])
            zb = singles.tile([128, n_steps], F32)
            nc.sync.dma_start(out=zb, in_=zb_d[:])
            dw = singles.tile([128, KC * VOCAB], BF16)
            nc.sync.dma_start(out=dw, in_=dw_d[:])
            ones = singles.tile([1, BQ], BF16)
            nc.vector.memset(ones, 1.0)
            zeros = singles.tile([128, BQ], BF16)
            nc.vector.memset(zeros, 0.0)

            # --- prefetch windows for per-step inputs ---
            # xh slots are separate 2D tiles: a 3D-slice operand knocks the
            # DVE add out of its 2x packed mode (measured 593ns vs 194ns)
            win = min(WIN, n_steps)
            pd = max(1, win // 2)
            xh_pool = ctx.enter_context(tc.tile_pool(name="xh", bufs=win))
            xh_slots = []
            oh_w = singles.tile([VOCAB, win, BQ], BF16)

            def xh_dma(t):
                xt = xh_pool.tile([128, 256], BF16, tag="xh", name=f"xh{t}")
                nc.sync.dma_start(out=xt, in_=xh_d[t])
                xh_slots.append(xt)

            for t in range(min(pd, n_steps)):
                xh_dma(t)
                nc.sync.dma_start(out=oh_w[:, t % win, :], in_=oh_d[t])

            # --- GRU state ---
            h_st = [
                state.tile([128, HQ], BF16, tag=f"h{i}", name=f"h{i}") for i in range(2)
            ]
            nc.vector.memset(h_st[0], 0.0)
            # per-step transposed h: even half (free cols 0:128 of h_new) and
            # odd half; chunk c stationary = hT_(c%2)[:, t, 32*(c//2):+32]
            hTe = state.tile([128, n_steps, 128], BF16, tag="hTe", name="hTe")
            hTo = state.tile([128, n_steps, 128], BF16, tag="hTo", name="hTo")

            def wzr_ap(k, j):
                return wzr[:, (k * Q + j) * 512 : (k * Q + j + 1) * 512]

            def wh_ap(k, j):
                return wh[:, (k * Q + j) * 256 : (k * Q + j + 1) * 256]

            # dense head: one chunk-matmul per recurrence step (uniform PE
            # fill in the chain gap); window w covers steps [w*DW, w*DW+DW)
            dstate = {"dps": None}

            def emit_dense_chunk(w, k):
                t0 = w * DW
                nsteps_w = min(DW, n_steps - t0)
                nfree = nsteps_w * BQ
                if k == 0:
                    dstate["dps"] = ps_d.tile(
                        [VOCAB, DW * BQ], F32, tag="dps", name=f"dps{w}"
                    )
                dps = dstate["dps"]
                src = hTe if k % 2 == 0 else hTo
                qq = k // 2
                nc.tensor.matmul(
                    dps[:, :nfree],
                    dw[:, k * VOCAB : (k + 1) * VOCAB],
                    src[:, t0 : t0 + nsteps_w, 32 * qq : 32 * (qq + 1)],
                    start=(k == 0),
                    stop=(k == KC - 1),
                )
                if k == KC - 1:
                    lg = outs.tile([VOCAB, DW * BQ], BF16, tag="lg")
                    if w % 2 == 0:
                        nc.scalar.copy(lg[:, :nfree], dps[:, :nfree])
                    else:
                        nc.vector.tensor_copy(lg[:, :nfree], dps[:, :nfree])
                    nc.sync.dma_start(
                        out=out_d[:, t0 : t0 + nsteps_w, :],
                        in_=lg[:, :nfree],
                    )

            def alloc_and_inject(t):
                # input injections for step t (no dependency on h_{t-1}):
                # emitted one step ahead so they fill the PE idle gap while
                # step t-1's elementwise chain runs
                zr_ps = ps_zr.tile([128, 512], F32, tag="zr", name=f"zr{t}")
                # full bank: cols 0:256 hold rh, cols 256:512 are a scratch
                # area for the HAM-warmer matmul
                rh_ps = ps_rh.tile([128, 512], F32, tag="rh", name=f"rh{t}")
                oh_t = oh_w[:, t % win, :]
                for j in range(Q):
                    nc.tensor.matmul(
                        zr_ps[32 * j : 32 * (j + 1), :],
                        oh_t,
                        gzr[:, j * 512 : (j + 1) * 512],
                        start=True,
                        stop=(t == 0),
                        tile_position=(0, 32 * j),
                    )
                for j in range(Q):
                    nc.tensor.matmul(
                        rh_ps[32 * j : 32 * (j + 1), :HQ],
                        ones,
                        b1h[:, j * 256 : (j + 1) * 256],
                        start=True,
                        stop=(t == 0),
                        tile_position=(0, 32 * j),
                    )
                return zr_ps, rh_ps

            prev_h = None  # h_new of previous step, pending transpose
            zr_cur, rh_cur = alloc_and_inject(0)
            CHUNKS = [0, 2, 4, 6, 1, 3, 5, 7]  # evens first (hTe copied first)

            for t in range(n_steps):
                h_prev = h_st[t % 2]
                h_new = h_st[(t + 1) % 2]
                zr_ps, rh_ps = zr_cur, rh_cur

                if t + pd < n_steps:
                    xh_dma(t + pd)
                    nc.sync.dma_start(out=oh_w[:, (t + pd) % win, :], in_=oh_d[t + pd])

                # --- deferred transpose of h_{t-1} into hTe/hTo[t-1] ---
                if prev_h is not None:
                    tr0 = ps_tr.tile([128, 128], BF16, tag="tr", name=f"tr{t}e")
                    nc.tensor.transpose(tr0, prev_h[:, :128], identb)
                    tr1 = ps_tr.tile([128, 128], BF16, tag="tr", name=f"tr{t}o")
                    nc.tensor.transpose(tr1, prev_h[:, 128:], identb)
                    nc.vector.tensor_copy(hTe[:, t - 1, :], tr0)
                    nc.vector.tensor_copy(hTo[:, t - 1, :], tr1)

                # --- recurrent weight streams ---
                if t > 0:
                    for ki, k in enumerate(CHUNKS):
                        src = hTe if k % 2 == 0 else hTo
                        hs = src[:, t - 1, 32 * (k // 2) : 32 * (k // 2 + 1)]
                        for j in range(Q):
                            nc.tensor.matmul(
                                zr_ps[32 * j : 32 * (j + 1), :],
                                hs,
                                wzr_ap(k, j),
                                start=False,
                                stop=(ki == KC - 1),
                                tile_position=(0, 32 * j),
                            )
                    for ki, k in enumerate(CHUNKS):
                        src = hTe if k % 2 == 0 else hTo
                        hs = src[:, t - 1, 32 * (k // 2) : 32 * (k // 2 + 1)]
                        for j in range(Q):
                            nc.tensor.matmul(
                                rh_ps[32 * j : 32 * (j + 1), :HQ],
                                hs,
                                wh_ap(k, j),
                                start=False,
                                stop=(ki == KC - 1),
                                tile_position=(0, 32 * j),
                            )

                # --- next step's injections: right after this stream on the
                # PE FIFO, so they run during this step's chain ---
                if t + 1 < n_steps:
                    zr_cur, rh_cur = alloc_and_inject(t + 1)

                # --- interleaved dense chunk (fills the PE idle gap) ---
                if t >= DW + 1:
                    w, k = (t - DW - 1) // DW, (t - DW - 1) % DW
                    if w < n_dw:
                        emit_dense_chunk(w, k)

                # --- elementwise chain ---
                # ACT: r, z sigmoids (run during the rh stream), then tanh halves
                r_t = temps.tile([128, HQ], BF16, tag="r")
                nc.scalar.activation(r_t, zr_ps[:, 256:512], AF.Sigmoid)
                z_t = temps.tile([128, HQ], BF16, tag="z")
                nc.scalar.activation(
                    z_t, zr_ps[:, :256], AF.Sigmoid, bias=zb[:, t : t + 1]
                )
                # GPSIMD: zc = 1 - z, zh = z * h_prev (off the DVE/ACT queues)
                zc_t = temps.tile([128, HQ], BF16, tag="zc")
                nc.gpsimd.tensor_scalar(
                    zc_t, z_t, -1.0, 1.0, mybir.AluOpType.mult, mybir.AluOpType.add
                )
                zh_t = temps.tile([128, HQ], BF16, tag="zh")
                nc.gpsimd.tensor_mul(zh_t, z_t, h_prev)
                # DVE: a1 = r*rh (psum), a2 = a1 + xh, then blend
                a1_t = temps.tile([128, HQ], BF16, tag="a1")
                nc.vector.tensor_mul(a1_t, r_t, rh_ps[:, :HQ])
                a2_t = temps.tile([128, HQ], BF16, tag="a2")
                nc.vector.tensor_add(a2_t, a1_t, xh_slots[t])
                hh_t = temps.tile([128, HQ], BF16, tag="hh")
                nc.scalar.activation(hh_t, a2_t, AF.Tanh)
                # HAM warmer: a zero-stationary accumulate (rh += 0*hh) gated
                # on hh lands mid-gap on the PE, keeping the clock at 8/8
                # through the chain window without touching the real sums.
                if t + 1 < n_steps:
                    nc.tensor.matmul(
                        rh_cur[0:BQ, :HQ],
                        zeros[:, 0:BQ],
                        hh_t,
                        start=False,
                        stop=False,
                        skip_group_check=True,
                    )
                u_t = temps.tile([128, HQ], BF16, tag="u")
                nc.vector.tensor_mul(u_t, zc_t, hh_t)
                nc.vector.tensor_add(h_new, u_t, zh_t)

                prev_h = h_new

            # --- epilogue: last transpose + remaining dense windows ---
            tr0 = ps_tr.tile([128, 128], BF16, tag="tr", name="trLe")
            nc.tensor.transpose(tr0, prev_h[:, :128], identb)
            tr1 = ps_tr.tile([128, 128], BF16, tag="tr", name="trLo")
            nc.tensor.transpose(tr1, prev_h[:, 128:], identb)
            nc.vector.tensor_copy(hTe[:, n_steps - 1, :], tr0)
            nc.vector.tensor_copy(hTo[:, n_steps - 1, :], tr1)

            for c in range(max(0, n_steps - DW - 1), n_dw * KC):
                emit_dense_chunk(c // KC, c % KC)

    _split_multiwaits(nc)
    return nc


# ---------------------------------------------------------------------------
# Host-side prep + run
# ---------------------------------------------------------------------------
_CACHE = {}


def _prep_inputs(x, padding_mask, emb_table, gru_kernel, gru_rec_kernel, gru_bias,
                 dense_w, dense_b, n_steps):
    x = np.asarray(x)
    padding_mask = np.asarray(padding_mask)
    emb_table = np.asarray(emb_table, dtype=np.float32)
    gru_kernel = np.asarray(gru_kernel, dtype=np.float32)
    W = np.asarray(gru_rec_kernel, dtype=np.float32)
    gru_bias = np.asarray(gru_bias, dtype=np.float32)
    dense_w = np.asarray(dense_w, dtype=np.float32)

    g = emb_table @ gru_kernel + gru_bias[0][None, :]   # [VOCAB, 3H], b0 folded
    g[:, : 2 * HID] += gru_bias[1][None, : 2 * HID]     # b1 folded for z, r
    gh = np.ascontiguousarray(g[:, 2 * HID :])          # [VOCAB, H] (b0h only)

    # fused [z|r] weight quarters: wzr[p, k, j, 0:256]=Wz, [256:512]=Wr
    Wz = W[:, :HID].reshape(HID, Q, HQ)
    Wr = W[:, HID : 2 * HID].reshape(HID, Q, HQ)
    wzr = np.concatenate([Wz, Wr], axis=2)              # [H, Q, 512]
    wzr = wzr.reshape(KC, 128, Q, 512).transpose(1, 0, 2, 3)
    whm = W[:, 2 * HID :].reshape(HID, Q, HQ)
    whm = whm.reshape(KC, 128, Q, HQ).transpose(1, 0, 2, 3)

    gz = g[:, :HID].reshape(VOCAB, Q, HQ)
    gr = g[:, HID : 2 * HID].reshape(VOCAB, Q, HQ)
    gzr = np.concatenate([gz, gr], axis=2)              # [VOCAB, Q, 512]

    b1h = gru_bias[1][2 * HID :].reshape(1, Q * HQ)

    dwp = dense_w.reshape(KC, 128, VOCAB).transpose(1, 0, 2)  # [128, KC, V]

    shared = {
        "wzr": np.ascontiguousarray(wzr.reshape(128, -1)).astype(ml_dtypes.bfloat16),
        "wh": np.ascontiguousarray(whm.reshape(128, -1)).astype(ml_dtypes.bfloat16),
        "gzr": np.ascontiguousarray(gzr.reshape(VOCAB, -1)).astype(ml_dtypes.bfloat16),
        "b1h": np.ascontiguousarray(b1h).astype(ml_dtypes.bfloat16),
        "identb": np.eye(128, dtype=np.float32).astype(ml_dtypes.bfloat16),
        "dw": np.ascontiguousarray(dwp.reshape(128, -1)).astype(ml_dtypes.bfloat16),
    }

    gh_bf = gh.astype(ml_dtypes.bfloat16)

    in_maps = []
    for c in range(N_CORES):
        q = c % Q
        xs = x[q * BQ : (q + 1) * BQ]                   # [BQ, T]
        ms = padding_mask[q * BQ : (q + 1) * BQ]

        # xh[t, p=32*qq+b, f] = gh[x[b,t], 256*qq + f]  (F-layout)
        e = gh_bf[xs[:, :n_steps]]                      # [BQ, T, H] bf16
        e = e.transpose(1, 0, 2).reshape(n_steps, BQ, Q, HQ)
        e = e.transpose(0, 2, 1, 3).reshape(n_steps, 128, HQ)

        onehot = np.zeros((n_steps, VOCAB, BQ), dtype=np.float32)
        tt = np.arange(n_steps)
        for b in range(BQ):
            onehot[tt, xs[b, :n_steps], b] = 1.0
        zbias = np.where(ms[:, :n_steps], 0.0, 30.0).astype(np.float32)  # [BQ, T]
        zbias = np.tile(zbias, (128 // BQ, 1))          # F-layout partitions
        in_maps.append(
            dict(
                shared,
                xh=np.ascontiguousarray(e),
                onehot=onehot.astype(ml_dtypes.bfloat16),
                zbias=np.ascontiguousarray(zbias),
            )
        )
    return in_maps


def kernel(x, padding_mask, emb_table, gru_kernel, gru_rec_kernel, gru_bias,
           dense_w, dense_b, _n_steps: int = T):
    from concourse.bass_utils import run_bass_kernel_spmd

    trace = os.environ.get("BASS_GRU_TRACE", "") == "1"
    if trace:
        _register_axon_profile_hook()

    n_steps = _n_steps
    if n_steps not in _CACHE:
        _CACHE[n_steps] = build_kernel(n_steps)
    nc = _CACHE[n_steps]

    in_maps = _prep_inputs(x, padding_mask, emb_table, gru_kernel, gru_rec_kernel,
                           gru_bias, dense_w, dense_b, n_steps)
    res = run_bass_kernel_spmd(nc, in_maps, list(range(N_CORES)), trace=trace)
    if trace:
        kernel.last_exec_time_ns = res.exec_time_ns
        print(f"HW exec time: {res.exec_time_ns} ns")

    db = np.asarray(dense_b, dtype=np.float32)
    out = np.empty((B, n_steps, VOCAB), dtype=np.float32)
    for q in range(Q):
        lg = np.asarray(res.results[q]["logits"], dtype=np.float32)  # [V, T, BQ]
        out[q * BQ : (q + 1) * BQ] = lg.transpose(2, 1, 0)
    out += db[None, None, :]
    return np.ascontiguousarray(out)


kernel.last_exec_time_ns = None


# revision 22
# speedup vs baseline: 1.2456x; 1.0192x over previous
"""Trainium2 Bass kernel for nn_CasparLayer (embedding -> GRU(reset_after) -> dense).

Problem shapes: B=128, T=256, VOCAB=41, EMB=512, HID=1024.

Strategy (per NeuronCore, SPMD x8, 4-way data parallel over batch):
  - Recurrent weight streaming on the PE with 4 column-tile groups
    (tile_position=(0,32j)), each holding the same stationary h chunk and
    streaming its own quarter of the weight columns. Per hidden chunk k the
    [z|r] sections stream as one fused N=512 matmul and the h section as
    N=256 (fewer instructions / LDWEIGHTS than per-gate MMs).
  - Embedding fused into the z/r stream as a one-hot K=41 matmul; the
    h-gate input projection xh = gcat_h[x] is precomputed on HOST and DMA'd
    (it sits under a DVE add, not a PE injection).
  - Keras masking folded into the gates: z' = sigmoid(zpre + 30*(1-m)) == 1
    on padded steps (h carries over); zc' = sigmoid(-zpre - 30*(1-m)) == 1-z'.
    Blend is h_new = zc*hh + z*h_prev (zc precomputed on ACT during stream).
  - h_new (F-layout [128,256]: partition=32*(h//256)+b) is PE-transposed in
    two 128x128 halves into per-step slots of two big SBUF buffers
    (hT_even/hT_odd) that double as the input to the dense head.
  - Dense head runs as batched weight-stationary GEMM windows (8 steps ->
    N=256 free) interleaved into the recurrence every 8 steps, filling the
    PE idle gap while the elementwise chain runs (keeps the HAM clock warm).

The harness contract: kernel(**inputs) takes full unsharded numpy inputs and
returns the full [128, 256, 41] float32 logits.
"""

import contextlib
import ctypes
import os
import sys
import types

sys.path.insert(0, "/opt/trn_rl_repo")

import numpy as np
import ml_dtypes

import bass_rust
import concourse.bass as bass
import concourse.tile as tile
from concourse import mybir

B = 128
T = 256
VOCAB = 41
EMB = 512
HID = 1024
H3 = 3 * HID
N_CORES = 8
BQ = 32   # batch quarter per core (4-way data parallel, x2 replicas)
Q = 4     # PE column groups = hidden quarters
KC = HID // 128  # 8 hidden-contraction chunks
HQ = HID // Q    # 256 columns per group section
WIN = 8   # xh/onehot SBUF prefetch window (steps)
DW = 8    # dense-head window (steps per batched GEMM)

F32 = mybir.dt.float32
BF16 = mybir.dt.bfloat16
AF = mybir.ActivationFunctionType


# ---------------------------------------------------------------------------
# Workaround: this walrus build accepts at most ONE sync wait per instruction;
# Tile attaches several. Hoist extras onto single-wait NOPs inserted before.
# ---------------------------------------------------------------------------
def _split_multiwaits(nc, max_waits: int = 1) -> int:
    n_split = 0
    for fn in nc.m.functions:
        for blk in fn.blocks:
            insts = blk.instructions
            i = 0
            while i < len(insts):
                ins = insts[i]
                si = ins.sync_info
                if si is not None and len(si.on_wait) > max_waits:
                    waits = list(si.on_wait)
                    keep = waits[-max_waits:]
                    hoist = waits[:-max_waits]
                    ins.sync_info = bass_rust.SyncInfo(
                        on_wait=keep, on_update=list(si.on_update)
                    )
                    for w in hoist:
                        nop = mybir.InstNoOp(
                            name=nc.get_next_instruction_name(),
                            sync_info=bass_rust.SyncInfo(on_wait=[w], on_update=[]),
                            bass_nofuse=True,
                            engine=ins.engine,
                            text_hint="wait_split",
                        )
                        nc.register_instruction(nop)
                        blk.instructions.insert(i, nop)
                        i += 1
                        n_split += 1
                i += 1
    return n_split


# ---------------------------------------------------------------------------
# Optional NTFF profiling under axon (the container's antenv stub lacks the
# hook registration module). Enabled via BASS_GRU_TRACE=1.
# ---------------------------------------------------------------------------
def _register_axon_profile_hook():
    so_path = "/opt/axon/libaxon_pjrt.so"
    if "antenv.axon_hooks" in sys.modules:
        return
    mod = types.ModuleType("antenv.axon_hooks")
    state = {"hook": None}
    mod.set_axon_ntff_profile_hook = lambda h: state.__setitem__("hook", h)
    mod.get_axon_ntff_profile_hook = lambda: state["hook"]
    sys.modules["antenv.axon_hooks"] = mod

    try:
        lib = ctypes.CDLL(so_path)
    except OSError:
        return
    if not hasattr(lib, "axon_start_nrt_profile"):
        return
    lib.axon_start_nrt_profile.argtypes = [
        ctypes.POINTER(ctypes.c_int64),
        ctypes.c_size_t,
    ]
    lib.axon_start_nrt_profile.restype = ctypes.c_int64
    lib.axon_stop_nrt_profile.argtypes = [ctypes.c_char_p]
    lib.axon_stop_nrt_profile.restype = ctypes.c_int64

    @contextlib.contextmanager
    def _hook_cm(output_dir, device_ids):
        import jax

        jax.devices()
        if device_ids:
            ids = (ctypes.c_int64 * len(device_ids))(*device_ids)
            rc = lib.axon_start_nrt_profile(ids, len(device_ids))
        else:
            rc = lib.axon_start_nrt_profile(None, 0)
        if rc != 0:
            raise RuntimeError(f"axon_start_nrt_profile rc={rc}")
        try:
            yield
        finally:
            n = lib.axon_stop_nrt_profile(str(output_dir).encode())
            print(f"ntff profile: {n} file(s) -> {output_dir}", file=sys.stderr)

    state["hook"] = _hook_cm

    import concourse.bass_utils as bu

    bu.upload_artifacts = lambda tmpdir: ""


# ---------------------------------------------------------------------------
# Kernel builder
# ---------------------------------------------------------------------------
def build_kernel(n_steps: int = T):
    nc = bass.Bass()

    wzr_d = nc.declare_dram_parameter("wzr", [128, KC * Q * 512], BF16, isOutput=False)
    wh_d = nc.declare_dram_parameter("wh", [128, KC * Q * 256], BF16, isOutput=False)
    gzr_d = nc.declare_dram_parameter("gzr", [VOCAB, Q * 512], BF16, isOutput=False)
    b1h_d = nc.declare_dram_parameter("b1h", [1, Q * 256], BF16, isOutput=False)
    xh_d = nc.declare_dram_parameter("xh", [n_steps, 128, 256], BF16, isOutput=False)
    oh_d = nc.declare_dram_parameter("onehot", [n_steps, VOCAB, BQ], BF16, isOutput=False)
    zb_d = nc.declare_dram_parameter("zbias", [128, n_steps], F32, isOutput=False)
    id_d = nc.declare_dram_parameter("identb", [128, 128], BF16, isOutput=False)
    dw_d = nc.declare_dram_parameter("dw", [128, KC * VOCAB], BF16, isOutput=False)
    out_d = nc.declare_dram_parameter("logits", [VOCAB, n_steps, BQ], BF16, isOutput=True)

    n_dw = (n_steps + DW - 1) // DW  # dense windows

    with tile.TileContext(nc) as tc:
        with contextlib.ExitStack() as ctx:
            singles = ctx.enter_context(tc.tile_pool(name="singles", bufs=1))
            state = ctx.enter_context(tc.tile_pool(name="state", bufs=1))
            temps = ctx.enter_context(tc.tile_pool(name="temps", bufs=2))
            outs = ctx.enter_context(tc.tile_pool(name="outs", bufs=2))
            ps_zr = ctx.enter_context(tc.tile_pool(name="ps_zr", bufs=2, space="PSUM"))
            ps_rh = ctx.enter_context(tc.tile_pool(name="ps_rh", bufs=2, space="PSUM"))
            ps_tr = ctx.enter_context(tc.tile_pool(name="ps_tr", bufs=2, space="PSUM"))
            ps_d = ctx.enter_context(tc.tile_pool(name="ps_d", bufs=2, space="PSUM"))

            # --- weights / constants resident in SBUF ---
            wzr = singles.tile([128, KC * Q * 512], BF16)
            nc.sync.dma_start(out=wzr, in_=wzr_d[:])
            wh = singles.tile([128, KC * Q * 256], BF16)
            nc.sync.dma_start(out=wh, in_=wh_d[:])
            gzr = singles.tile([VOCAB, Q * 512], BF16)
            nc.sync.dma_start(out=gzr, in_=gzr_d[:])
            b1h = singles.tile([1, Q * 256], BF16)
            nc.sync.dma_start(out=b1h, in_=b1h_d[:])
            identb = singles.tile([128, 128], BF16)
            nc.sync.dma_start(out=identb, in_=id_d[:])
            zb = singles.tile([128, n_steps], F32)
            nc.sync.dma_start(out=zb, in_=zb_d[:])
            zbn = singles.tile([128, n_steps], F32)
            nc.vector.tensor_scalar_mul(zbn, zb, -1.0)
            dw = singles.tile([128, KC * VOCAB], BF16)
            nc.sync.dma_start(out=dw, in_=dw_d[:])
            ones = singles.tile([1, BQ], BF16)
            nc.vector.memset(ones, 1.0)
            zeros = singles.tile([128, BQ], BF16)
            nc.vector.memset(zeros, 0.0)

            # --- prefetch windows for per-step inputs ---
            # xh slots are separate 2D tiles: a 3D-slice operand knocks the
            # DVE add out of its 2x packed mode (measured 593ns vs 194ns)
            win = min(WIN, n_steps)
            pd = max(1, win // 2)
            xh_pool = ctx.enter_context(tc.tile_pool(name="xh", bufs=win))
            xh_slots = []
            oh_w = singles.tile([VOCAB, win, BQ], BF16)

            def xh_dma(t):
                xt = xh_pool.tile([128, 256], BF16, tag="xh", name=f"xh{t}")
                nc.sync.dma_start(out=xt, in_=xh_d[t])
                xh_slots.append(xt)

            for t in range(min(pd, n_steps)):
                xh_dma(t)
                nc.sync.dma_start(out=oh_w[:, t % win, :], in_=oh_d[t])

            # --- GRU state ---
            h_st = [
                state.tile([128, HQ], BF16, tag=f"h{i}", name=f"h{i}") for i in range(2)
            ]
            nc.vector.memset(h_st[0], 0.0)
            # per-step transposed h: even half (free cols 0:128 of h_new) and
            # odd half; chunk c stationary = hT_(c%2)[:, t, 32*(c//2):+32]
            hTe = state.tile([128, n_steps, 128], BF16, tag="hTe", name="hTe")
            hTo = state.tile([128, n_steps, 128], BF16, tag="hTo", name="hTo")

            def wzr_ap(k, j):
                return wzr[:, (k * Q + j) * 512 : (k * Q + j + 1) * 512]

            def wh_ap(k, j):
                return wh[:, (k * Q + j) * 256 : (k * Q + j + 1) * 256]

            # dense head: one chunk-matmul per recurrence step (uniform PE
            # fill in the chain gap); window w covers steps [w*DW, w*DW+DW)
            dstate = {"dps": None}

            def emit_dense_chunk(w, k):
                t0 = w * DW
                nsteps_w = min(DW, n_steps - t0)
                nfree = nsteps_w * BQ
                if k == 0:
                    dstate["dps"] = ps_d.tile(
                        [VOCAB, DW * BQ], F32, tag="dps", name=f"dps{w}"
                    )
                dps = dstate["dps"]
                src = hTe if k % 2 == 0 else hTo
                qq = k // 2
                nc.tensor.matmul(
                    dps[:, :nfree],
                    dw[:, k * VOCAB : (k + 1) * VOCAB],
                    src[:, t0 : t0 + nsteps_w, 32 * qq : 32 * (qq + 1)],
                    start=(k == 0),
                    stop=(k == KC - 1),
                )
                if k == KC - 1:
                    lg = outs.tile([VOCAB, DW * BQ], BF16, tag="lg")
                    if w % 2 == 0:
                        nc.scalar.copy(lg[:, :nfree], dps[:, :nfree])
                    else:
                        nc.vector.tensor_copy(lg[:, :nfree], dps[:, :nfree])
                    nc.sync.dma_start(
                        out=out_d[:, t0 : t0 + nsteps_w, :],
                        in_=lg[:, :nfree],
                    )

            def alloc_and_inject(t):
                # input injections for step t (no dependency on h_{t-1}):
                # emitted one step ahead so they fill the PE idle gap while
                # step t-1's elementwise chain runs
                zr_ps = ps_zr.tile([128, 512], F32, tag="zr", name=f"zr{t}")
                # full bank: cols 0:256 hold rh, cols 256:512 are a scratch
                # area for the HAM-warmer matmul
                rh_ps = ps_rh.tile([128, 512], F32, tag="rh", name=f"rh{t}")
                oh_t = oh_w[:, t % win, :]
                for j in range(Q):
                    nc.tensor.matmul(
                        zr_ps[32 * j : 32 * (j + 1), :],
                        oh_t,
                        gzr[:, j * 512 : (j + 1) * 512],
                        start=True,
                        stop=(t == 0),
                        tile_position=(0, 32 * j),
                    )
                for j in range(Q):
                    nc.tensor.matmul(
                        rh_ps[32 * j : 32 * (j + 1), :HQ],
                        ones,
                        b1h[:, j * 256 : (j + 1) * 256],
                        start=True,
                        stop=(t == 0),
                        tile_position=(0, 32 * j),
                    )
                return zr_ps, rh_ps

            prev_h = None  # h_new of previous step, pending transpose
            zr_cur, rh_cur = alloc_and_inject(0)
            CHUNKS = [0, 2, 4, 6, 1, 3, 5, 7]  # evens first (hTe copied first)

            for t in range(n_steps):
                h_prev = h_st[t % 2]
                h_new = h_st[(t + 1) % 2]
                zr_ps, rh_ps = zr_cur, rh_cur

                if t + pd < n_steps:
                    xh_dma(t + pd)
                    nc.sync.dma_start(out=oh_w[:, (t + pd) % win, :], in_=oh_d[t + pd])

                # --- deferred transpose of h_{t-1} into hTe/hTo[t-1] ---
                if prev_h is not None:
                    tr0 = ps_tr.tile([128, 128], BF16, tag="tr", name=f"tr{t}e")
                    nc.tensor.transpose(tr0, prev_h[:, :128], identb)
                    tr1 = ps_tr.tile([128, 128], BF16, tag="tr", name=f"tr{t}o")
                    nc.tensor.transpose(tr1, prev_h[:, 128:], identb)
                    nc.vector.tensor_copy(hTe[:, t - 1, :], tr0)
                    nc.vector.tensor_copy(hTo[:, t - 1, :], tr1)

                # --- recurrent weight streams ---
                if t > 0:
                    for ki, k in enumerate(CHUNKS):
                        src = hTe if k % 2 == 0 else hTo
                        hs = src[:, t - 1, 32 * (k // 2) : 32 * (k // 2 + 1)]
                        for j in range(Q):
                            nc.tensor.matmul(
                                zr_ps[32 * j : 32 * (j + 1), :],
                                hs,
                                wzr_ap(k, j),
                                start=False,
                                stop=(ki == KC - 1),
                                tile_position=(0, 32 * j),
                            )
                    for ki, k in enumerate(CHUNKS):
                        src = hTe if k % 2 == 0 else hTo
                        hs = src[:, t - 1, 32 * (k // 2) : 32 * (k // 2 + 1)]
                        for j in range(Q):
                            nc.tensor.matmul(
                                rh_ps[32 * j : 32 * (j + 1), :HQ],
                                hs,
                                wh_ap(k, j),
                                start=False,
                                stop=(ki == KC - 1),
                                tile_position=(0, 32 * j),
                            )

                # --- next step's injections: right after this stream on the
                # PE FIFO, so they run during this step's chain ---
                if t + 1 < n_steps:
                    zr_cur, rh_cur = alloc_and_inject(t + 1)

                # --- interleaved dense chunk (fills the PE idle gap) ---
                if t >= DW + 1:
                    w, k = (t - DW - 1) // DW, (t - DW - 1) % DW
                    if w < n_dw:
                        emit_dense_chunk(w, k)

                # --- elementwise chain ---
                # ACT: r, z sigmoids (run during the rh stream), then tanh halves
                r_t = temps.tile([128, HQ], BF16, tag="r")
                nc.scalar.activation(r_t, zr_ps[:, 256:512], AF.Sigmoid)
                z_t = temps.tile([128, HQ], BF16, tag="z")
                nc.scalar.activation(
                    z_t, zr_ps[:, :256], AF.Sigmoid, bias=zb[:, t : t + 1]
                )
                # zc = 1-z via sigmoid(-x) = 1-sigmoid(x) (ACT, off-critical)
                zc_t = temps.tile([128, HQ], BF16, tag="zc")
                nc.scalar.activation(
                    zc_t, zr_ps[:, :256], AF.Sigmoid, scale=-1.0,
                    bias=zbn[:, t : t + 1],
                )

                # HAM warmers: zero-stationary accumulates (rh += 0*x) gated
                # on successive chain results spread PE activity through the
                # chain window, keeping the clock at 8/8. Numerically no-ops.
                def warm(x):
                    if t + 1 < n_steps:
                        nc.tensor.matmul(
                            rh_cur[0:BQ, :HQ],
                            zeros[:, 0:BQ],
                            x,
                            start=False,
                            stop=False,
                            skip_group_check=True,
                        )

                # DVE: a1 = r*rh (psum), a2 = a1 + xh, zh = z*h_prev, blend
                a1_t = temps.tile([128, HQ], BF16, tag="a1")
                nc.vector.tensor_mul(a1_t, r_t, rh_ps[:, :HQ])
                warm(a1_t)
                a2_t = temps.tile([128, HQ], BF16, tag="a2")
                nc.vector.tensor_add(a2_t, a1_t, xh_slots[t])
                warm(a2_t)
                hh_t = temps.tile([128, HQ], BF16, tag="hh")
                nc.scalar.activation(hh_t, a2_t, AF.Tanh)
                warm(hh_t)
                zh_t = temps.tile([128, HQ], BF16, tag="zh")
                nc.vector.tensor_mul(zh_t, z_t, h_prev)
                u_t = temps.tile([128, HQ], BF16, tag="u")
                nc.vector.tensor_mul(u_t, zc_t, hh_t)
                warm(u_t)
                nc.vector.tensor_add(h_new, u_t, zh_t)

                prev_h = h_new

            # --- epilogue: last transpose + remaining dense windows ---
            tr0 = ps_tr.tile([128, 128], BF16, tag="tr", name="trLe")
            nc.tensor.transpose(tr0, prev_h[:, :128], identb)
            tr1 = ps_tr.tile([128, 128], BF16, tag="tr", name="trLo")
            nc.tensor.transpose(tr1, prev_h[:, 128:], identb)
            nc.vector.tensor_copy(hTe[:, n_steps - 1, :], tr0)
            nc.vector.tensor_copy(hTo[:, n_steps - 1, :], tr1)

            for c in range(max(0, n_steps - DW - 1), n_dw * KC):
                emit_dense_chunk(c // KC, c % KC)

    _split_multiwaits(nc)
    return nc


# ---------------------------------------------------------------------------
# Host-side prep + run
# ---------------------------------------------------------------------------
_CACHE = {}


def _prep_inputs(x, padding_mask, emb_table, gru_kernel, gru_rec_kernel, gru_bias,
                 dense_w, dense_b, n_steps):
    x = np.asarray(x)
    padding_mask = np.asarray(padding_mask)
    emb_table = np.asarray(emb_table, dtype=np.float32)
    gru_kernel = np.asarray(gru_kernel, dtype=np.float32)
    W = np.asarray(gru_rec_kernel, dtype=np.float32)
    gru_bias = np.asarray(gru_bias, dtype=np.float32)
    dense_w = np.asarray(dense_w, dtype=np.float32)

    g = emb_table @ gru_kernel + gru_bias[0][None, :]   # [VOCAB, 3H], b0 folded
    g[:, : 2 * HID] += gru_bias[1][None, : 2 * HID]     # b1 folded for z, r
    gh = np.ascontiguousarray(g[:, 2 * HID :])          # [VOCAB, H] (b0h only)

    # fused [z|r] weight quarters: wzr[p, k, j, 0:256]=Wz, [256:512]=Wr
    Wz = W[:, :HID].reshape(HID, Q, HQ)
    Wr = W[:, HID : 2 * HID].reshape(HID, Q, HQ)
    wzr = np.concatenate([Wz, Wr], axis=2)              # [H, Q, 512]
    wzr = wzr.reshape(KC, 128, Q, 512).transpose(1, 0, 2, 3)
    whm = W[:, 2 * HID :].reshape(HID, Q, HQ)
    whm = whm.reshape(KC, 128, Q, HQ).transpose(1, 0, 2, 3)

    gz = g[:, :HID].reshape(VOCAB, Q, HQ)
    gr = g[:, HID : 2 * HID].reshape(VOCAB, Q, HQ)
    gzr = np.concatenate([gz, gr], axis=2)              # [VOCAB, Q, 512]

    b1h = gru_bias[1][2 * HID :].reshape(1, Q * HQ)

    dwp = dense_w.reshape(KC, 128, VOCAB).transpose(1, 0, 2)  # [128, KC, V]

    shared = {
        "wzr": np.ascontiguousarray(wzr.reshape(128, -1)).astype(ml_dtypes.bfloat16),
        "wh": np.ascontiguousarray(whm.reshape(128, -1)).astype(ml_dtypes.bfloat16),
        "gzr": np.ascontiguousarray(gzr.reshape(VOCAB, -1)).astype(ml_dtypes.bfloat16),
        "b1h": np.ascontiguousarray(b1h).astype(ml_dtypes.bfloat16),
        "identb": np.eye(128, dtype=np.float32).astype(ml_dtypes.bfloat16),
        "dw": np.ascontiguousarray(dwp.reshape(128, -1)).astype(ml_dtypes.bfloat16),
    }

    gh_bf = gh.astype(ml_dtypes.bfloat16)

    in_maps = []
    for c in range(N_CORES):
        q = c % Q
        xs = x[q * BQ : (q + 1) * BQ]                   # [BQ, T]
        ms = padding_mask[q * BQ : (q + 1) * BQ]

        # xh[t, p=32*qq+b, f] = gh[x[b,t], 256*qq + f]  (F-layout)
        e = gh_bf[xs[:, :n_steps]]                      # [BQ, T, H] bf16
        e = e.transpose(1, 0, 2).reshape(n_steps, BQ, Q, HQ)
        e = e.transpose(0, 2, 1, 3).reshape(n_steps, 128, HQ)

        onehot = np.zeros((n_steps, VOCAB, BQ), dtype=np.float32)
        tt = np.arange(n_steps)
        for b in range(BQ):
            onehot[tt, xs[b, :n_steps], b] = 1.0
        zbias = np.where(ms[:, :n_steps], 0.0, 30.0).astype(np.float32)  # [BQ, T]
        zbias = np.tile(zbias, (128 // BQ, 1))          # F-layout partitions
        in_maps.append(
            dict(
                shared,
                xh=np.ascontiguousarray(e),
                onehot=onehot.astype(ml_dtypes.bfloat16),
                zbias=np.ascontiguousarray(zbias),
            )
        )
    return in_maps


def kernel(x, padding_mask, emb_table, gru_kernel, gru_rec_kernel, gru_bias,
           dense_w, dense_b, _n_steps: int = T):
    from concourse.bass_utils import run_bass_kernel_spmd

    trace = os.environ.get("BASS_GRU_TRACE", "") == "1"
    if trace:
        _register_axon_profile_hook()

    n_steps = _n_steps
    if n_steps not in _CACHE:
        _CACHE[n_steps] = build_kernel(n_steps)
    nc = _CACHE[n_steps]

    in_maps = _prep_inputs(x, padding_mask, emb_table, gru_kernel, gru_rec_kernel,
                           gru_bias, dense_w, dense_b, n_steps)
    res = run_bass_kernel_spmd(nc, in_maps, list(range(N_CORES)), trace=trace)
    if trace:
        kernel.last_exec_time_ns = res.exec_time_ns
        print(f"HW exec time: {res.exec_time_ns} ns")

    db = np.asarray(dense_b, dtype=np.float32)
    out = np.empty((B, n_steps, VOCAB), dtype=np.float32)
    for q in range(Q):
        lg = np.asarray(res.results[q]["logits"], dtype=np.float32)  # [V, T, BQ]
        out[q * BQ : (q + 1) * BQ] = lg.transpose(2, 1, 0)
    out += db[None, None, :]
    return np.ascontiguousarray(out)


kernel.last_exec_time_ns = None
